# revision 25
# baseline (speedup 1.0000x reference)
"""Distributed Trainium2 Bass kernel for masked multi-head attention.

Problem: out = (softmax(scale * x Wq^T (x Wk^T)^T + mask * -1e5) (x Wv^T)) Wp^T + bp
  x [4, 2048, 768], mask [4, 2048, 2048], H=12 heads, D=64.

Sharding (8 cores): core = (batch b, head-group hg) with b = core//2,
hg = core%2 (6 heads each).  Column-parallel Wq/Wkv, row-parallel Wp;
each core produces a partial [2048, 768] output; the host sums the two
head-group partials per batch and adds the bias, then stacks batches.

Device schedule: the ACT engine's exp stream (192 tiles x ~1.03us) is
the pace setter; everything is arranged so it starts early and rarely
waits:
  - Minimal prefix (Q/K projection of head-pair 0 / block 0 on
    fine-grained x^T DMA) so the first QK->exp fires at ~8us instead of
    after a 60us serial projection phase.
  - All other projections drip into the attention iteration stream as
    PE filler.  Their PSUM comes from the shared "ot" slots, kept free
    by deferring early PV matmuls (pm tiles queue in a deep SBUF pool
    and drain after the per-qchunk O accumulators allocate); overflow
    units borrow "ring" slots once the otiles exist.
  - qchunk boundaries: previous epilogue (softmax divide / transpose /
    out-projection) and next q-block projections run in a deferred-PV
    window at the top of each qchunk; divisions are emitted first in
    DVE program order so the otile slots free immediately.
  - Engines: ACT = exp only; DVE = mask-mul + divisions + even-head
    evictions; Pool = odd-head evictions, psum->sbuf copies, mask DMA
    issue; SP = x/w/out DMA.
  - qt/kt pack head PAIRS on partitions (rows 0:64 even head, 64:128
    odd head): QK contracts over 64 rows directly, evictions stay on
    their own partitions, SBUF halves.
"""

import os
from collections import deque
from contextlib import ExitStack

import ml_dtypes
import numpy as np

import sys
import types

try:  # defensive: concourse's trace path imports this; absent on some images
    import antenv.axon_hooks  # noqa: F401
except ImportError:
    try:
        import antenv
        _m = types.ModuleType('antenv.axon_hooks')
        _m._hook = None
        _m.set_axon_ntff_profile_hook = lambda h: setattr(_m, '_hook', h)
        _m.get_axon_ntff_profile_hook = lambda: _m._hook
        sys.modules['antenv.axon_hooks'] = _m
        antenv.axon_hooks = _m
    except ImportError:
        pass

import concourse.bass as bass
import concourse.tile as tile
from concourse import bacc, mybir
from concourse.bass_utils import run_bass_kernel_spmd
from concourse.masks import make_identity

B, N, C, H, D = 4, 2048, 768, 12, 64
SCALE = D ** -0.5
NCORES = 8
HGROUPS = 2
HL = H // HGROUPS          # 6 heads per group
CH = HL * D                # 384 channels per group
P = 128
NKT = N // P               # 16 k tiles
QCHUNK = 512
NQC = N // QCHUNK          # 4 q chunks
QSUBS = QCHUNK // P        # 4
CIN_T = C // P             # 6 input-channel tiles
CH_T = CH // P             # 3 group-channel tiles
MP = HL // 2               # 3 head pairs per group
E = D + 1                  # head slot width in O psum (64 V cols + 1 ones col)
NIT = HL * NKT // 2        # 48 iterations per qchunk

F32 = mybir.dt.float32
BF16 = mybir.dt.bfloat16


def build_kernel():
    nc = bacc.Bacc("TRN2", target_bir_lowering=False, debug=False,
                   num_devices=NCORES)

    xT = nc.dram_tensor("xT", [C, N], BF16, kind="ExternalInput").ap()
    wqt = nc.dram_tensor("wqt", [C, CH], BF16, kind="ExternalInput").ap()
    wkt = nc.dram_tensor("wkt", [C, CH], BF16, kind="ExternalInput").ap()
    wvt = nc.dram_tensor("wvt", [C, CH], BF16, kind="ExternalInput").ap()
    wpt = nc.dram_tensor("wpt", [CH, C], BF16, kind="ExternalInput").ap()
    negmt = nc.dram_tensor("negmt", [N, N], BF16, kind="ExternalInput").ap()
    out = nc.dram_tensor("out", [N, C], F32, kind="ExternalOutput").ap()

    with tile.TileContext(nc) as tc, ExitStack() as ctx:
        persist = ctx.enter_context(tc.tile_pool(name="persist", bufs=1))
        ring_pool = ctx.enter_context(
            tc.tile_pool(name="ring", bufs=2, space="PSUM"))
        o_pool = ctx.enter_context(
            tc.tile_pool(name="opsum", bufs=4, space="PSUM"))

        # head-pair packed Q^T / K^T: rows 0:64 head 2m, rows 64:128 head 2m+1
        qt_sb = [persist.tile([P, N], BF16, tag=f"qt{m}", name=f"qt{m}")
                 for m in range(MP)]
        kt_sb = [persist.tile([P, N], BF16, tag=f"kt{m}", name=f"kt{m}")
                 for m in range(MP)]
        vp_sb = [persist.tile([P, HL, E], BF16, tag=f"vp{j}", name=f"vp{j}")
                 for j in range(NKT)]
        wp_sb = [persist.tile([P, C], BF16, tag=f"wp{t}", name=f"wp{t}")
                 for t in range(CH_T)]
        idn = persist.tile([P, P], BF16, tag="idn")

        ph1 = ctx.enter_context(tc.tile_pool(name="ph1", bufs=1))
        xt_sb = [ph1.tile([P, N], BF16, tag=f"xt{i}", name=f"xt{i}")
                 for i in range(CIN_T)]
        wq_sb = [ph1.tile([P, CH], BF16, tag=f"wq{i}", name=f"wq{i}")
                 for i in range(CIN_T)]
        wk_sb = [ph1.tile([P, CH], BF16, tag=f"wk{i}", name=f"wk{i}")
                 for i in range(CIN_T)]
        wv_sb = [ph1.tile([P, CH], BF16, tag=f"wv{i}", name=f"wv{i}")
                 for i in range(CIN_T)]

        # ---- DMA issue.  Pool queue issues in ~25ns (vs ~1us SWDGE on
        # SP), so everything the first ~15us depends on goes there, in
        # consumption order; SP carries the later x^T blocks + wp + out.
        mpool = ctx.enter_context(tc.tile_pool(name="mask", bufs=2))
        mk_tiles = {}

        def issue_mask(qc):
            q0 = qc * QCHUNK
            mk = mpool.tile([P, NKT, QCHUNK], BF16, tag="mk", name=f"mk{qc}")
            for j in range(NKT):
                nc.gpsimd.dma_start(
                    out=mk[:, j, :],
                    in_=negmt[j * P:(j + 1) * P, q0:q0 + QCHUNK])
            mk_tiles[qc] = mk

        for i in range(CIN_T):
            sl = slice(i * P, (i + 1) * P)
            nc.gpsimd.dma_start(out=wq_sb[i][:, 0:P], in_=wqt[sl, 0:P])
            nc.gpsimd.dma_start(out=wk_sb[i][:, 0:P], in_=wkt[sl, 0:P])
        for i in range(CIN_T):
            nc.gpsimd.dma_start(out=xt_sb[i][:, 0:QCHUNK],
                                in_=xT[i * P:(i + 1) * P, 0:QCHUNK])
        for i in range(CIN_T):
            sl = slice(i * P, (i + 1) * P)
            nc.gpsimd.dma_start(out=wv_sb[i], in_=wvt[sl, :])
        issue_mask(0)
        for i in range(CIN_T):
            sl = slice(i * P, (i + 1) * P)
            nc.gpsimd.dma_start(out=wq_sb[i][:, P:CH], in_=wqt[sl, P:CH])
            nc.gpsimd.dma_start(out=wk_sb[i][:, P:CH], in_=wkt[sl, P:CH])
        issue_mask(1)
        for nck in range(1, NQC):
            q0 = nck * QCHUNK
            for i in range(CIN_T):
                nc.sync.dma_start(out=xt_sb[i][:, q0:q0 + QCHUNK],
                                  in_=xT[i * P:(i + 1) * P, q0:q0 + QCHUNK])
        for t in range(CH_T):
            nc.sync.dma_start(out=wp_sb[t], in_=wpt[t * P:(t + 1) * P, :])

        # ---- pointwise pools ----
        p_pool = ctx.enter_context(tc.tile_pool(name="pexp", bufs=3))
        pm_pool = ctx.enter_context(tc.tile_pool(name="pmask", bufs=12))
        epi = ctx.enter_context(tc.tile_pool(name="epi", bufs=6))
        otsb_pool = ctx.enter_context(tc.tile_pool(name="otsb", bufs=4))
        outsb_pool = ctx.enter_context(tc.tile_pool(name="outsb", bufs=2))

        # ---- projection units (one psum slot + eviction each) ----
        v_emitted = set()

        def unit_q(m, n, w_sb, dst):
            ps = o_pool.tile([P, QCHUNK], F32, tag="ot", name=f"pj{m}_{n}")
            for ci in range(CIN_T):
                nc.tensor.matmul(
                    ps,
                    w_sb[ci][:, m * P:(m + 1) * P],
                    xt_sb[ci][:, n * QCHUNK:(n + 1) * QCHUNK],
                    start=(ci == 0), stop=(ci == CIN_T - 1))
            dt = dst[m]
            nsl = slice(n * QCHUNK, (n + 1) * QCHUNK)
            # both packed heads in one copy (GPSIMD cannot read PSUM)
            nc.vector.tensor_copy(dt[:, nsl], ps)

        def unit_v(j):
            ps = o_pool.tile([P, CH], F32, tag="ot", name=f"vps{j}")
            for ci in range(CIN_T):
                nc.tensor.matmul(
                    ps,
                    xt_sb[ci][:, j * P:(j + 1) * P],
                    wv_sb[ci],
                    start=(ci == 0), stop=(ci == CIN_T - 1))
            nc.gpsimd.memset(vp_sb[j], 1.0)
            nc.vector.tensor_copy(
                vp_sb[j][:, :, 0:D],
                ps.rearrange("p (h d) -> p h d", h=HL))
            v_emitted.add(j)

        # ---- attention iteration pieces ----
        # iterations come in pairs: two QK+exp (2 k-tiles each) fill one
        # [P, 4, QCHUNK] pexp tile; one mask-mul + one 4-k-tile PV group
        # per pair (halves the DVE instruction count).
        pvq = deque()          # deferred PV groups: (h, g, pm, otiles)
        cur_pexp = [None]

        def emit_qk_exp(qc, h, ktp, otiles):
            m, r = divmod(h, 2)
            rows = slice(r * D, (r + 1) * D)
            q0 = qc * QCHUNK
            half = ktp % 2
            ring = ring_pool.tile([P, 2, QCHUNK], F32, tag="ring",
                                  name=f"ring{qc}_{h}_{ktp}")
            for u in range(2):
                kti = 2 * ktp + u
                nc.tensor.matmul(
                    ring[:, u, :],
                    kt_sb[m][rows, kti * P:(kti + 1) * P],
                    qt_sb[m][rows, q0:q0 + QCHUNK],
                    start=True, stop=True)
            if half == 0:
                cur_pexp[0] = p_pool.tile([P, 4, QCHUNK], BF16, tag="pe",
                                          name=f"pe{qc}_{h}_{ktp}")
            pexp = cur_pexp[0]
            nc.scalar.activation(
                pexp[:, 2 * half:2 * half + 2, :], ring,
                mybir.ActivationFunctionType.Exp)
            if half == 1:
                g = ktp // 2           # 4-k-tile group within the head
                pm = pm_pool.tile([P, 4, QCHUNK], BF16, tag="pm",
                                  name=f"pm{qc}_{h}_{g}")
                nc.vector.tensor_mul(
                    pm, pexp, mk_tiles[qc][:, 4 * g:4 * g + 4, :])
                pvq.append((h, g, pm, otiles))

        def front_ready(qc):
            if qc > 0:
                return True
            g = pvq[0][1]
            return all(4 * g + u in v_emitted for u in range(4))

        # ---- softmax divisions, eager per head-range: heads 0..4 divide
        # as soon as their PV chains complete (mid-qchunk); only head 5
        # remains on the post-stream critical path ----
        osb_store = {}

        def emit_divisions(qc, otiles, h0, nh):
            for s in range(QSUBS):
                otv = otiles[s].rearrange("p (h e) -> p h e", h=HL)
                zrec = epi.tile([P, nh], F32, tag=f"zr{nh}",
                                name=f"zr{qc}_{s}_{h0}")
                nc.vector.reciprocal(zrec, otv[:, h0:h0 + nh, D])
                osb = osb_store.get((qc, s))
                if osb is None:
                    osb = epi.tile([P, HL, D], BF16, tag="osb", bufs=8,
                                   name=f"osb{qc}_{s}")
                    osb_store[(qc, s)] = osb
                zb = bass.AP(
                    tensor=zrec.tensor, offset=zrec.offset,
                    ap=[*zrec.ap, [0, D]])
                nc.vector.tensor_mul(
                    osb[:, h0:h0 + nh, :], otv[:, h0:h0 + nh, 0:D], zb)

        def emit_pv_group(qc):
            h, g, pm, otiles = pvq.popleft()
            for u in range(4):
                kti = 4 * g + u
                for s in range(QSUBS):
                    nc.tensor.matmul(
                        otiles[s][:, h * E:(h + 1) * E],
                        pm[:, u, s * P:(s + 1) * P],
                        vp_sb[kti][:, h, :],
                        start=(kti == 0), stop=(kti == NKT - 1))
            if g == NKT // 4 - 1:
                if h == HL - 2:
                    emit_divisions(qc, otiles, 0, HL - 1)
                elif h == HL - 1:
                    emit_divisions(qc, otiles, HL - 1, 1)

        def epi_stile(qc, s, last=False):
            q0 = qc * QCHUNK
            osf = osb_store[(qc, s)].rearrange("p h d -> p (h d)")
            otp = o_pool.tile([P, CH_T, P], BF16, tag="ot",
                              name=f"otp{qc}_{s}")
            for ct in range(CH_T):
                nc.tensor.transpose(
                    otp[:, ct, :], osf[:, ct * P:(ct + 1) * P], idn)
            otsb = otsb_pool.tile([P, CH_T, P], BF16, tag="otsb",
                                  name=f"otsb{qc}_{s}")
            # ACT: lands in the PE-bound boundary window where exp idles
            nc.scalar.copy(otsb, otp)
            ppss = []
            for cf, (c0, c1) in enumerate(((0, CH), (CH, C))):
                pps = o_pool.tile([P, CH], F32, tag="ot",
                                  name=f"pps{qc}_{s}_{cf}")
                for ct in range(CH_T):
                    nc.tensor.matmul(
                        pps,
                        otsb[:, ct, :],
                        wp_sb[ct][:, c0:c1],
                        start=(ct == 0), stop=(ct == CH_T - 1))
                ppss.append(pps)
            ob = outsb_pool.tile([P, C], F32, tag="ob", name=f"ob{qc}_{s}")
            if last:
                nc.scalar.copy(ob[:, 0:CH], ppss[0])
                nc.vector.tensor_copy(ob[:, CH:C], ppss[1])
            else:
                nc.vector.tensor_copy(ob[:, 0:CH], ppss[0])
                nc.vector.tensor_copy(ob[:, CH:C], ppss[1])
            nc.sync.dma_start(
                out=out[q0 + s * P:q0 + (s + 1) * P, :], in_=ob)

        # ---- emission ----
        # prefix: Q/K projections for head pair 0, q/k block 0
        unit_q(0, 0, wq_sb, qt_sb)
        unit_q(0, 0, wk_sb, kt_sb)
        make_identity(nc, idn)

        def run_qchunk(qc, window, window_units, drain_rate):
            otiles = None
            wi = 0
            drain_credit = 0.0
            for it in range(NIT):
                h, ktp = divmod(it, NKT // 2)
                if it == window:
                    otiles = [o_pool.tile([P, HL * E], F32, tag="ot",
                                          name=f"otile{qc}_{s_}")
                              for s_ in range(QSUBS)]
                    old = [(g[0], g[1], g[2]) for g in pvq]
                    pvq.clear()
                    for gh, gg, gpm in old:
                        pvq.append((gh, gg, gpm, otiles))
                emit_qk_exp(qc, h, ktp, otiles)
                if it < window:
                    due = ((it + 1) * len(window_units) + window - 1) // window
                    while wi < min(due, len(window_units)):
                        window_units[wi]()
                        wi += 1
                else:
                    drain_credit = min(drain_credit + drain_rate, 3.0)
                    while (drain_credit >= 1.0 and len(pvq) > 1
                           and front_ready(qc)):
                        emit_pv_group(qc)
                        drain_credit -= 1.0
            while pvq:
                emit_pv_group(qc)
            return otiles

        pending = None
        for qc in range(NQC):
            if qc == 0:
                # 32 units: V(16), K(0,1..3), K(1,*), K(2,*), Q(1,0),
                # Q(2,0), Q(*,1).  Deadlines: K(0,n) before iter 2n;
                # K/Q(1,*) before iter 16; K/Q(2,*) before iter 32;
                # V(j) before the h0 drain; Q(*,1) before qc1.
                window = 24
                window_units = (
                    [lambda: unit_v(0),
                     lambda: unit_q(0, 1, wk_sb, kt_sb),
                     lambda: unit_v(1),
                     lambda: unit_q(0, 2, wk_sb, kt_sb),
                     lambda: unit_v(2),
                     lambda: unit_q(0, 3, wk_sb, kt_sb),
                     lambda: unit_v(3),
                     lambda: unit_v(4),
                     lambda: unit_q(1, 0, wq_sb, qt_sb)]
                    + [lambda n=n: unit_q(1, n, wk_sb, kt_sb)
                       for n in range(NQC)]
                    + [lambda j=j: unit_v(j) for j in (5, 6, 7, 8)]
                    + [lambda: unit_q(2, 0, wq_sb, qt_sb)]
                    + [lambda n=n: unit_q(2, n, wk_sb, kt_sb)
                       for n in range(NQC)]
                    + [lambda j=j: unit_v(j) for j in range(9, NKT)]
                    + [lambda m=m: unit_q(m, 1, wq_sb, qt_sb)
                       for m in range(MP)])
                drain_rate = 1.0
            else:
                window = 16
                if qc + 1 < NQC:
                    issue_mask(qc + 1)
                pqc = pending
                window_units = []
                for s in range(QSUBS):
                    window_units.append(
                        lambda s=s, p=pqc: epi_stile(p, s))
                    if qc < NQC - 1 and s < MP:
                        window_units.append(
                            lambda m=s, n=qc + 1: unit_q(m, n, wq_sb, qt_sb))
                drain_rate = 0.8
            run_qchunk(qc, window, window_units, drain_rate)
            pending = qc

        # ---- final epilogue: stage-parallel across the four s-tiles so
        # the post-exp tail is short; ACT and DVE split the copies ----
        pqc = pending
        otps, otsbs = [], []
        for s in range(QSUBS):
            osf = osb_store[(pqc, s)].rearrange("p h d -> p (h d)")
            otp = o_pool.tile([P, CH_T, P], BF16, tag="ot", name=f"fotp{s}")
            for ct in range(CH_T):
                nc.tensor.transpose(
                    otp[:, ct, :], osf[:, ct * P:(ct + 1) * P], idn)
            otps.append(otp)
            otsb = otsb_pool.tile([P, CH_T, P], BF16, tag="otsb",
                                  name=f"fotsb{s}")
            if s % 2 == 0:
                nc.scalar.copy(otsb, otp)
            else:
                nc.vector.tensor_copy(otsb, otp)
            otsbs.append(otsb)
        for s in range(QSUBS):
            ppss = []
            for cf, (c0, c1) in enumerate(((0, CH), (CH, C))):
                pps = o_pool.tile([P, CH], F32, tag="ot", name=f"fpps{s}_{cf}")
                for ct in range(CH_T):
                    nc.tensor.matmul(
                        pps, otsbs[s][:, ct, :], wp_sb[ct][:, c0:c1],
                        start=(ct == 0), stop=(ct == CH_T - 1))
                ppss.append(pps)
            ob = outsb_pool.tile([P, C], F32, tag="ob", name=f"fob{s}")
            nc.scalar.copy(ob[:, 0:CH], ppss[0])
            nc.vector.tensor_copy(ob[:, CH:C], ppss[1])
            nc.sync.dma_start(
                out=out[pqc * QCHUNK + s * P:pqc * QCHUNK + (s + 1) * P, :],
                in_=ob)

    nc.compile()
    return nc


_CACHE = {}


def _get_nc():
    if "nc" not in _CACHE:
        _CACHE["nc"] = build_kernel()
    return _CACHE["nc"]


def kernel(x, mask, Wq, Wkv, Wp, bp):
    x = np.asarray(x, np.float32)
    mask = np.asarray(mask, np.float32)
    Wq = np.asarray(Wq, np.float32)
    Wkv = np.asarray(Wkv, np.float32)
    Wp = np.asarray(Wp, np.float32)
    bp = np.asarray(bp, np.float32)

    nc = _get_nc()
    in_maps = []
    for core in range(NCORES):
        b, hg = divmod(core, HGROUPS)
        rows = slice(hg * CH, (hg + 1) * CH)
        in_maps.append({
            "xT": np.ascontiguousarray(x[b].T.astype(ml_dtypes.bfloat16)),
            "wqt": np.ascontiguousarray(((Wq[rows, :] * SCALE).T).astype(ml_dtypes.bfloat16)),
            "wkt": np.ascontiguousarray(Wkv[rows, :].T.astype(ml_dtypes.bfloat16)),
            "wvt": np.ascontiguousarray(Wkv.T[:, C + hg * CH:C + (hg + 1) * CH].astype(ml_dtypes.bfloat16)),
            "wpt": np.ascontiguousarray(Wp[:, rows].T.astype(ml_dtypes.bfloat16)),
            "negmt": np.ascontiguousarray(
                (1.0 - mask[b].T).astype(ml_dtypes.bfloat16)),
        })

    trace = os.environ.get("KERNEL_TRACE", "0") == "1"
    if os.environ.get("KERNEL_WARMUP", "1") == "1":
        run_bass_kernel_spmd(nc, in_maps, core_ids=list(range(NCORES)),
                             trace=False)
    res = run_bass_kernel_spmd(nc, in_maps, core_ids=list(range(NCORES)),
                               trace=trace)
    kernel.last_results = res

    outs = [res.results[i]["out"] for i in range(NCORES)]
    full = np.empty((B, N, C), np.float32)
    for b in range(B):
        full[b] = outs[2 * b] + outs[2 * b + 1] + bp[None, :]
    return full


# revision 28
# speedup vs baseline: 1.0380x; 1.0380x over previous
"""Distributed Trainium2 Bass kernel for masked multi-head attention.

Problem: out = (softmax(scale * x Wq^T (x Wk^T)^T + mask * -1e5) (x Wv^T)) Wp^T + bp
  x [4, 2048, 768], mask [4, 2048, 2048], H=12 heads, D=64.

Sharding (8 cores): core = (batch b, head-group hg) with b = core//2,
hg = core%2 (6 heads each).  Column-parallel Wq/Wkv, row-parallel Wp;
each core produces a partial [2048, 768] output; the host sums the two
head-group partials per batch and adds the bias, then stacks batches.

Device schedule: the ACT engine's exp stream (192 tiles x ~1.03us) is
the pace setter; everything is arranged so it starts early and rarely
waits:
  - Minimal prefix (Q/K projection of head-pair 0 / block 0 on
    fine-grained x^T DMA) so the first QK->exp fires at ~8us instead of
    after a 60us serial projection phase.
  - All other projections drip into the attention iteration stream as
    PE filler.  Their PSUM comes from the shared "ot" slots, kept free
    by deferring early PV matmuls (pm tiles queue in a deep SBUF pool
    and drain after the per-qchunk O accumulators allocate); overflow
    units borrow "ring" slots once the otiles exist.
  - qchunk boundaries: previous epilogue (softmax divide / transpose /
    out-projection) and next q-block projections run in a deferred-PV
    window at the top of each qchunk; divisions are emitted first in
    DVE program order so the otile slots free immediately.
  - Engines: ACT = exp only; DVE = mask-mul + divisions + even-head
    evictions; Pool = odd-head evictions, psum->sbuf copies, mask DMA
    issue; SP = x/w/out DMA.
  - qt/kt pack head PAIRS on partitions (rows 0:64 even head, 64:128
    odd head): QK contracts over 64 rows directly, evictions stay on
    their own partitions, SBUF halves.
"""

import os
from collections import deque
from contextlib import ExitStack

import ml_dtypes
import numpy as np

import sys
import types

try:  # defensive: concourse's trace path imports this; absent on some images
    import antenv.axon_hooks  # noqa: F401
except ImportError:
    try:
        import antenv
        _m = types.ModuleType('antenv.axon_hooks')
        _m._hook = None
        _m.set_axon_ntff_profile_hook = lambda h: setattr(_m, '_hook', h)
        _m.get_axon_ntff_profile_hook = lambda: _m._hook
        sys.modules['antenv.axon_hooks'] = _m
        antenv.axon_hooks = _m
    except ImportError:
        pass

import concourse.bass as bass
import concourse.tile as tile
from concourse import bacc, mybir
from concourse.bass_utils import run_bass_kernel_spmd
from concourse.masks import make_identity

B, N, C, H, D = 4, 2048, 768, 12, 64
SCALE = D ** -0.5
NCORES = 8
HGROUPS = 2
HL = H // HGROUPS          # 6 heads per group
CH = HL * D                # 384 channels per group
P = 128
NKT = N // P               # 16 k tiles
QCHUNK = 512
NQC = N // QCHUNK          # 4 q chunks
QSUBS = QCHUNK // P        # 4
CIN_T = C // P             # 6 input-channel tiles
CH_T = CH // P             # 3 group-channel tiles
MP = HL // 2               # 3 head pairs per group
E = D + 1                  # head slot width in O psum (64 V cols + 1 ones col)
NIT = HL * NKT // 2        # 48 iterations per qchunk

F32 = mybir.dt.float32
BF16 = mybir.dt.bfloat16


def build_kernel():
    nc = bacc.Bacc("TRN2", target_bir_lowering=False, debug=False,
                   num_devices=NCORES)

    xT = nc.dram_tensor("xT", [C, N], BF16, kind="ExternalInput").ap()
    wqt = nc.dram_tensor("wqt", [C, CH], BF16, kind="ExternalInput").ap()
    wkt = nc.dram_tensor("wkt", [C, CH], BF16, kind="ExternalInput").ap()
    wvt = nc.dram_tensor("wvt", [C, CH], BF16, kind="ExternalInput").ap()
    wpt = nc.dram_tensor("wpt", [CH, C], BF16, kind="ExternalInput").ap()
    negmt = nc.dram_tensor("negmt", [N, N], BF16, kind="ExternalInput").ap()
    out = nc.dram_tensor("out", [N, C], F32, kind="ExternalOutput").ap()

    with tile.TileContext(nc) as tc, ExitStack() as ctx:
        persist = ctx.enter_context(tc.tile_pool(name="persist", bufs=1))
        ring_pool = ctx.enter_context(
            tc.tile_pool(name="ring", bufs=2, space="PSUM"))
        o_pool = ctx.enter_context(
            tc.tile_pool(name="opsum", bufs=4, space="PSUM"))

        # head-pair packed Q^T / K^T: rows 0:64 head 2m, rows 64:128 head 2m+1
        qt_sb = [persist.tile([P, N], BF16, tag=f"qt{m}", name=f"qt{m}")
                 for m in range(MP)]
        kt_sb = [persist.tile([P, N], BF16, tag=f"kt{m}", name=f"kt{m}")
                 for m in range(MP)]
        vp_sb = [persist.tile([P, HL, E], BF16, tag=f"vp{j}", name=f"vp{j}")
                 for j in range(NKT)]
        wp_sb = [persist.tile([P, C], BF16, tag=f"wp{t}", name=f"wp{t}")
                 for t in range(CH_T)]
        idn = persist.tile([P, P], BF16, tag="idn")

        ph1 = ctx.enter_context(tc.tile_pool(name="ph1", bufs=1))
        xt_all = ph1.tile([P, CIN_T, N], BF16, tag="xt", name="xt")
        wq_all = ph1.tile([P, CIN_T, CH], BF16, tag="wq", name="wq")
        wk_all = ph1.tile([P, CIN_T, CH], BF16, tag="wk", name="wk")
        wv_all = ph1.tile([P, CIN_T, CH], BF16, tag="wv", name="wv")

        # ---- DMA issue.  Each dma_start costs ~1us on its issuing
        # engine, so inputs are packed tensors with ONE transfer each;
        # masks are one transfer per qchunk (Pool queue), critical
        # prefix bytes (wq/wk pair-0 columns, x^T block 0) first.
        mpool = ctx.enter_context(tc.tile_pool(name="mask", bufs=2))
        mk_tiles = {}

        def issue_mask(qc):
            q0 = qc * QCHUNK
            mk = mpool.tile([P, NKT, QCHUNK], BF16, tag="mk", name=f"mk{qc}")
            nc.gpsimd.dma_start(
                out=mk,
                in_=negmt[:, q0:q0 + QCHUNK].rearrange(
                    "(j p) q -> p j q", p=P))
            mk_tiles[qc] = mk

        issue_mask(0)
        issue_mask(1)
        nc.sync.dma_start(
            out=wq_all[:, :, 0:P],
            in_=wqt[:, 0:P].rearrange("(ci p) c -> p ci c", p=P))
        nc.sync.dma_start(
            out=wk_all[:, :, 0:P],
            in_=wkt[:, 0:P].rearrange("(ci p) c -> p ci c", p=P))
        nc.sync.dma_start(
            out=xt_all[:, :, 0:QCHUNK],
            in_=xT[:, 0:QCHUNK].rearrange("(ci p) n -> p ci n", p=P))
        nc.sync.dma_start(
            out=wv_all, in_=wvt.rearrange("(ci p) c -> p ci c", p=P))
        nc.sync.dma_start(
            out=wq_all[:, :, P:CH],
            in_=wqt[:, P:CH].rearrange("(ci p) c -> p ci c", p=P))
        nc.sync.dma_start(
            out=wk_all[:, :, P:CH],
            in_=wkt[:, P:CH].rearrange("(ci p) c -> p ci c", p=P))
        nc.sync.dma_start(
            out=xt_all[:, :, QCHUNK:N],
            in_=xT[:, QCHUNK:N].rearrange("(ci p) n -> p ci n", p=P))
        for t in range(CH_T):
            nc.sync.dma_start(out=wp_sb[t], in_=wpt[t * P:(t + 1) * P, :])

        # ---- pointwise pools ----
        p_pool = ctx.enter_context(tc.tile_pool(name="pexp", bufs=3))
        pm_pool = ctx.enter_context(tc.tile_pool(name="pmask", bufs=12))
        epi = ctx.enter_context(tc.tile_pool(name="epi", bufs=6))
        otsb_pool = ctx.enter_context(tc.tile_pool(name="otsb", bufs=4))
        outsb_pool = ctx.enter_context(tc.tile_pool(name="outsb", bufs=2))

        # ---- projection units (one psum slot + eviction each) ----
        v_emitted = set()

        def unit_q(m, n, w_all, dst):
            ps = o_pool.tile([P, QCHUNK], F32, tag="ot", name=f"pj{m}_{n}")
            for ci in range(CIN_T):
                nc.tensor.matmul(
                    ps,
                    w_all[:, ci, m * P:(m + 1) * P],
                    xt_all[:, ci, n * QCHUNK:(n + 1) * QCHUNK],
                    start=(ci == 0), stop=(ci == CIN_T - 1))
            dt = dst[m]
            nsl = slice(n * QCHUNK, (n + 1) * QCHUNK)
            # both packed heads in one copy (GPSIMD cannot read PSUM)
            nc.vector.tensor_copy(dt[:, nsl], ps)

        def unit_v(j):
            ps = o_pool.tile([P, CH], F32, tag="ot", name=f"vps{j}")
            for ci in range(CIN_T):
                nc.tensor.matmul(
                    ps,
                    xt_all[:, ci, j * P:(j + 1) * P],
                    wv_all[:, ci, :],
                    start=(ci == 0), stop=(ci == CIN_T - 1))
            nc.gpsimd.memset(vp_sb[j], 1.0)
            nc.vector.tensor_copy(
                vp_sb[j][:, :, 0:D],
                ps.rearrange("p (h d) -> p h d", h=HL))
            v_emitted.add(j)

        # ---- attention iteration pieces ----
        # iterations come in pairs: two QK+exp (2 k-tiles each) fill one
        # [P, 4, QCHUNK] pexp tile; one mask-mul + one 4-k-tile PV group
        # per pair (halves the DVE instruction count).
        pvq = deque()          # deferred PV groups: (h, g, pm, otiles)
        cur_pexp = [None]

        def emit_qk_exp(qc, h, ktp, otiles):
            m, r = divmod(h, 2)
            rows = slice(r * D, (r + 1) * D)
            q0 = qc * QCHUNK
            half = ktp % 2
            ring = ring_pool.tile([P, 2, QCHUNK], F32, tag="ring",
                                  name=f"ring{qc}_{h}_{ktp}")
            for u in range(2):
                kti = 2 * ktp + u
                nc.tensor.matmul(
                    ring[:, u, :],
                    kt_sb[m][rows, kti * P:(kti + 1) * P],
                    qt_sb[m][rows, q0:q0 + QCHUNK],
                    start=True, stop=True)
            if half == 0:
                cur_pexp[0] = p_pool.tile([P, 4, QCHUNK], BF16, tag="pe",
                                          name=f"pe{qc}_{h}_{ktp}")
            pexp = cur_pexp[0]
            nc.scalar.activation(
                pexp[:, 2 * half:2 * half + 2, :], ring,
                mybir.ActivationFunctionType.Exp)
            if half == 1:
                g = ktp // 2           # 4-k-tile group within the head
                pm = pm_pool.tile([P, 4, QCHUNK], BF16, tag="pm",
                                  name=f"pm{qc}_{h}_{g}")
                nc.vector.tensor_mul(
                    pm, pexp, mk_tiles[qc][:, 4 * g:4 * g + 4, :])
                pvq.append((h, g, pm, otiles))

        def front_ready(qc):
            if qc > 0:
                return True
            g = pvq[0][1]
            return all(4 * g + u in v_emitted for u in range(4))

        # ---- softmax divisions, eager per head-range: heads 0..4 divide
        # as soon as their PV chains complete (mid-qchunk); only head 5
        # remains on the post-stream critical path ----
        osb_store = {}

        def emit_divisions(qc, otiles, h0, nh):
            for s in range(QSUBS):
                otv = otiles[s].rearrange("p (h e) -> p h e", h=HL)
                zrec = epi.tile([P, nh], F32, tag=f"zr{nh}",
                                name=f"zr{qc}_{s}_{h0}")
                nc.vector.reciprocal(zrec, otv[:, h0:h0 + nh, D])
                osb = osb_store.get((qc, s))
                if osb is None:
                    osb = epi.tile([P, HL, D], BF16, tag="osb", bufs=8,
                                   name=f"osb{qc}_{s}")
                    osb_store[(qc, s)] = osb
                zb = bass.AP(
                    tensor=zrec.tensor, offset=zrec.offset,
                    ap=[*zrec.ap, [0, D]])
                nc.vector.tensor_mul(
                    osb[:, h0:h0 + nh, :], otv[:, h0:h0 + nh, 0:D], zb)

        def emit_pv_group(qc):
            h, g, pm, otiles = pvq.popleft()
            for u in range(4):
                kti = 4 * g + u
                for s in range(QSUBS):
                    nc.tensor.matmul(
                        otiles[s][:, h * E:(h + 1) * E],
                        pm[:, u, s * P:(s + 1) * P],
                        vp_sb[kti][:, h, :],
                        start=(kti == 0), stop=(kti == NKT - 1))
            if g == NKT // 4 - 1:
                if h == HL - 2:
                    emit_divisions(qc, otiles, 0, HL - 1)
                elif h == HL - 1:
                    emit_divisions(qc, otiles, HL - 1, 1)

        def epi_stile(qc, s, last=False):
            q0 = qc * QCHUNK
            osf = osb_store[(qc, s)].rearrange("p h d -> p (h d)")
            otp = o_pool.tile([P, CH_T, P], BF16, tag="ot",
                              name=f"otp{qc}_{s}")
            for ct in range(CH_T):
                nc.tensor.transpose(
                    otp[:, ct, :], osf[:, ct * P:(ct + 1) * P], idn)
            otsb = otsb_pool.tile([P, CH_T, P], BF16, tag="otsb",
                                  name=f"otsb{qc}_{s}")
            # ACT: lands in the PE-bound boundary window where exp idles
            nc.scalar.copy(otsb, otp)
            ppss = []
            for cf, (c0, c1) in enumerate(((0, CH), (CH, C))):
                pps = o_pool.tile([P, CH], F32, tag="ot",
                                  name=f"pps{qc}_{s}_{cf}")
                for ct in range(CH_T):
                    nc.tensor.matmul(
                        pps,
                        otsb[:, ct, :],
                        wp_sb[ct][:, c0:c1],
                        start=(ct == 0), stop=(ct == CH_T - 1))
                ppss.append(pps)
            ob = outsb_pool.tile([P, C], F32, tag="ob", name=f"ob{qc}_{s}")
            if last:
                nc.scalar.copy(ob[:, 0:CH], ppss[0])
                nc.vector.tensor_copy(ob[:, CH:C], ppss[1])
            else:
                nc.vector.tensor_copy(ob[:, 0:CH], ppss[0])
                nc.vector.tensor_copy(ob[:, CH:C], ppss[1])
            nc.sync.dma_start(
                out=out[q0 + s * P:q0 + (s + 1) * P, :], in_=ob)

        # ---- emission ----
        # prefix: Q/K projections for head pair 0, q/k block 0
        unit_q(0, 0, wq_all, qt_sb)
        unit_q(0, 0, wk_all, kt_sb)
        make_identity(nc, idn)

        def run_qchunk(qc, window, window_units, drain_rate):
            otiles = None
            wi = 0
            drain_credit = 0.0
            for it in range(NIT):
                h, ktp = divmod(it, NKT // 2)
                if it == window:
                    otiles = [o_pool.tile([P, HL * E], F32, tag="ot",
                                          name=f"otile{qc}_{s_}")
                              for s_ in range(QSUBS)]
                    old = [(g[0], g[1], g[2]) for g in pvq]
                    pvq.clear()
                    for gh, gg, gpm in old:
                        pvq.append((gh, gg, gpm, otiles))
                emit_qk_exp(qc, h, ktp, otiles)
                if it < window:
                    due = ((it + 1) * len(window_units) + window - 1) // window
                    while wi < min(due, len(window_units)):
                        window_units[wi]()
                        wi += 1
                else:
                    drain_credit = min(drain_credit + drain_rate, 3.0)
                    while (drain_credit >= 1.0 and len(pvq) > 1
                           and front_ready(qc)):
                        emit_pv_group(qc)
                        drain_credit -= 1.0
            while pvq:
                emit_pv_group(qc)
            return otiles

        pending = None
        for qc in range(NQC):
            if qc == 0:
                # 32 units: V(16), K(0,1..3), K(1,*), K(2,*), Q(1,0),
                # Q(2,0), Q(*,1).  Deadlines: K(0,n) before iter 2n;
                # K/Q(1,*) before iter 16; K/Q(2,*) before iter 32;
                # V(j) before the h0 drain; Q(*,1) before qc1.
                window = 24
                window_units = (
                    [lambda: unit_v(0),
                     lambda: unit_q(0, 1, wk_all, kt_sb),
                     lambda: unit_v(1),
                     lambda: unit_q(0, 2, wk_all, kt_sb),
                     lambda: unit_v(2),
                     lambda: unit_q(0, 3, wk_all, kt_sb),
                     lambda: unit_v(3),
                     lambda: unit_v(4),
                     lambda: unit_q(1, 0, wq_all, qt_sb)]
                    + [lambda n=n: unit_q(1, n, wk_all, kt_sb)
                       for n in range(NQC)]
                    + [lambda j=j: unit_v(j) for j in (5, 6, 7, 8)]
                    + [lambda: unit_q(2, 0, wq_all, qt_sb)]
                    + [lambda n=n: unit_q(2, n, wk_all, kt_sb)
                       for n in range(NQC)]
                    + [lambda j=j: unit_v(j) for j in range(9, NKT)]
                    + [lambda m=m: unit_q(m, 1, wq_all, qt_sb)
                       for m in range(MP)])
                drain_rate = 1.0
            else:
                window = 16
                if qc + 1 < NQC:
                    issue_mask(qc + 1)
                pqc = pending
                window_units = []
                for s in range(QSUBS):
                    window_units.append(
                        lambda s=s, p=pqc: epi_stile(p, s))
                    if qc < NQC - 1 and s < MP:
                        window_units.append(
                            lambda m=s, n=qc + 1: unit_q(m, n, wq_all, qt_sb))
                drain_rate = 0.8
            run_qchunk(qc, window, window_units, drain_rate)
            pending = qc

        # ---- final epilogue: stage-parallel across the four s-tiles so
        # the post-exp tail is short; ACT and DVE split the copies ----
        pqc = pending
        otps, otsbs = [], []
        for s in range(QSUBS):
            osf = osb_store[(pqc, s)].rearrange("p h d -> p (h d)")
            otp = o_pool.tile([P, CH_T, P], BF16, tag="ot", name=f"fotp{s}")
            for ct in range(CH_T):
                nc.tensor.transpose(
                    otp[:, ct, :], osf[:, ct * P:(ct + 1) * P], idn)
            otps.append(otp)
            otsb = otsb_pool.tile([P, CH_T, P], BF16, tag="otsb",
                                  name=f"fotsb{s}")
            if s % 2 == 0:
                nc.scalar.copy(otsb, otp)
            else:
                nc.vector.tensor_copy(otsb, otp)
            otsbs.append(otsb)
        for s in range(QSUBS):
            ppss = []
            for cf, (c0, c1) in enumerate(((0, CH), (CH, C))):
                pps = o_pool.tile([P, CH], F32, tag="ot", name=f"fpps{s}_{cf}")
                for ct in range(CH_T):
                    nc.tensor.matmul(
                        pps, otsbs[s][:, ct, :], wp_sb[ct][:, c0:c1],
                        start=(ct == 0), stop=(ct == CH_T - 1))
                ppss.append(pps)
            ob = outsb_pool.tile([P, C], F32, tag="ob", name=f"fob{s}")
            nc.scalar.copy(ob[:, 0:CH], ppss[0])
            nc.vector.tensor_copy(ob[:, CH:C], ppss[1])
            nc.sync.dma_start(
                out=out[pqc * QCHUNK + s * P:pqc * QCHUNK + (s + 1) * P, :],
                in_=ob)

    nc.compile()
    return nc


_CACHE = {}


def _get_nc():
    if "nc" not in _CACHE:
        _CACHE["nc"] = build_kernel()
    return _CACHE["nc"]


def kernel(x, mask, Wq, Wkv, Wp, bp):
    x = np.asarray(x, np.float32)
    mask = np.asarray(mask, np.float32)
    Wq = np.asarray(Wq, np.float32)
    Wkv = np.asarray(Wkv, np.float32)
    Wp = np.asarray(Wp, np.float32)
    bp = np.asarray(bp, np.float32)

    nc = _get_nc()
    in_maps = []
    for core in range(NCORES):
        b, hg = divmod(core, HGROUPS)
        rows = slice(hg * CH, (hg + 1) * CH)
        in_maps.append({
            "xT": np.ascontiguousarray(x[b].T.astype(ml_dtypes.bfloat16)),
            "wqt": np.ascontiguousarray(((Wq[rows, :] * SCALE).T).astype(ml_dtypes.bfloat16)),
            "wkt": np.ascontiguousarray(Wkv[rows, :].T.astype(ml_dtypes.bfloat16)),
            "wvt": np.ascontiguousarray(Wkv.T[:, C + hg * CH:C + (hg + 1) * CH].astype(ml_dtypes.bfloat16)),
            "wpt": np.ascontiguousarray(Wp[:, rows].T.astype(ml_dtypes.bfloat16)),
            "negmt": np.ascontiguousarray(
                (1.0 - mask[b].T).astype(ml_dtypes.bfloat16)),
        })

    trace = os.environ.get("KERNEL_TRACE", "0") == "1"
    if os.environ.get("KERNEL_WARMUP", "1") == "1":
        run_bass_kernel_spmd(nc, in_maps, core_ids=list(range(NCORES)),
                             trace=False)
    res = run_bass_kernel_spmd(nc, in_maps, core_ids=list(range(NCORES)),
                               trace=trace)
    kernel.last_results = res

    outs = [res.results[i]["out"] for i in range(NCORES)]
    full = np.empty((B, N, C), np.float32)
    for b in range(B):
        full[b] = outs[2 * b] + outs[2 * b + 1] + bp[None, :]
    return full


# revision 30
# speedup vs baseline: 1.0692x; 1.0301x over previous
"""Distributed Trainium2 Bass kernel for masked multi-head attention.

Problem: out = (softmax(scale * x Wq^T (x Wk^T)^T + mask * -1e5) (x Wv^T)) Wp^T + bp
  x [4, 2048, 768], mask [4, 2048, 2048], H=12 heads, D=64.

Sharding (8 cores): core = (batch b, head-group hg) with b = core//2,
hg = core%2 (6 heads each).  Column-parallel Wq/Wkv, row-parallel Wp;
each core produces a partial [2048, 768] output; the host sums the two
head-group partials per batch and adds the bias, then stacks batches.

Device schedule: the ACT engine's exp stream (192 tiles x ~1.03us) is
the pace setter; everything is arranged so it starts early and rarely
waits:
  - Minimal prefix (Q/K projection of head-pair 0 / block 0 on
    fine-grained x^T DMA) so the first QK->exp fires at ~8us instead of
    after a 60us serial projection phase.
  - All other projections drip into the attention iteration stream as
    PE filler.  Their PSUM comes from the shared "ot" slots, kept free
    by deferring early PV matmuls (pm tiles queue in a deep SBUF pool
    and drain after the per-qchunk O accumulators allocate); overflow
    units borrow "ring" slots once the otiles exist.
  - qchunk boundaries: previous epilogue (softmax divide / transpose /
    out-projection) and next q-block projections run in a deferred-PV
    window at the top of each qchunk; divisions are emitted first in
    DVE program order so the otile slots free immediately.
  - Engines: ACT = exp only; DVE = mask-mul + divisions + even-head
    evictions; Pool = odd-head evictions, psum->sbuf copies, mask DMA
    issue; SP = x/w/out DMA.
  - qt/kt pack head PAIRS on partitions (rows 0:64 even head, 64:128
    odd head): QK contracts over 64 rows directly, evictions stay on
    their own partitions, SBUF halves.
"""

import os
from collections import deque
from contextlib import ExitStack

import ml_dtypes
import numpy as np

import sys
import types

try:  # defensive: concourse's trace path imports this; absent on some images
    import antenv.axon_hooks  # noqa: F401
except ImportError:
    try:
        import antenv
        _m = types.ModuleType('antenv.axon_hooks')
        _m._hook = None
        _m.set_axon_ntff_profile_hook = lambda h: setattr(_m, '_hook', h)
        _m.get_axon_ntff_profile_hook = lambda: _m._hook
        sys.modules['antenv.axon_hooks'] = _m
        antenv.axon_hooks = _m
    except ImportError:
        pass

import concourse.bass as bass
import concourse.tile as tile
from concourse import bacc, mybir
from concourse.bass_utils import run_bass_kernel_spmd
from concourse.masks import make_identity

B, N, C, H, D = 4, 2048, 768, 12, 64
SCALE = D ** -0.5
NCORES = 8
HGROUPS = 2
HL = H // HGROUPS          # 6 heads per group
CH = HL * D                # 384 channels per group
P = 128
NKT = N // P               # 16 k tiles
QCHUNK = 512
NQC = N // QCHUNK          # 4 q chunks
QSUBS = QCHUNK // P        # 4
CIN_T = C // P             # 6 input-channel tiles
CH_T = CH // P             # 3 group-channel tiles
MP = HL // 2               # 3 head pairs per group
E = D + 1                  # head slot width in O psum (64 V cols + 1 ones col)
NIT = HL * NKT // 2        # 48 iterations per qchunk

F32 = mybir.dt.float32
BF16 = mybir.dt.bfloat16


def build_kernel():
    nc = bacc.Bacc("TRN2", target_bir_lowering=False, debug=False,
                   num_devices=NCORES)

    xT = nc.dram_tensor("xT", [C, N], BF16, kind="ExternalInput").ap()
    wqt = nc.dram_tensor("wqt", [C, CH], BF16, kind="ExternalInput").ap()
    wkt = nc.dram_tensor("wkt", [C, CH], BF16, kind="ExternalInput").ap()
    wvt = nc.dram_tensor("wvt", [C, CH], BF16, kind="ExternalInput").ap()
    wpt = nc.dram_tensor("wpt", [CH, C], BF16, kind="ExternalInput").ap()
    negmt = nc.dram_tensor("negmt", [N, N], BF16, kind="ExternalInput").ap()
    out = nc.dram_tensor("out", [N, C], F32, kind="ExternalOutput").ap()

    with tile.TileContext(nc) as tc, ExitStack() as ctx:
        persist = ctx.enter_context(tc.tile_pool(name="persist", bufs=1))
        ring_pool = ctx.enter_context(
            tc.tile_pool(name="ring", bufs=2, space="PSUM"))
        o_pool = ctx.enter_context(
            tc.tile_pool(name="opsum", bufs=4, space="PSUM"))

        # head-pair packed Q^T / K^T: rows 0:64 head 2m, rows 64:128 head 2m+1
        qt_sb = [persist.tile([P, N], BF16, tag=f"qt{m}", name=f"qt{m}")
                 for m in range(MP)]
        kt_sb = [persist.tile([P, N], BF16, tag=f"kt{m}", name=f"kt{m}")
                 for m in range(MP)]
        vp_sb = [persist.tile([P, HL, E], BF16, tag=f"vp{j}", name=f"vp{j}")
                 for j in range(NKT)]
        wp_all = persist.tile([P, CH_T, C], BF16, tag="wp", name="wp")
        idn = persist.tile([P, P], BF16, tag="idn")

        ph1 = ctx.enter_context(tc.tile_pool(name="ph1", bufs=1))
        xt_all = ph1.tile([P, CIN_T, N], BF16, tag="xt", name="xt")
        wq_all = ph1.tile([P, CIN_T, CH], BF16, tag="wq", name="wq")
        wk_all = ph1.tile([P, CIN_T, CH], BF16, tag="wk", name="wk")
        wv_all = ph1.tile([P, CIN_T, CH], BF16, tag="wv", name="wv")

        # ---- DMA issue.  Each dma_start costs ~1us on its issuing
        # engine, so inputs are packed tensors with ONE transfer each;
        # masks are one transfer per qchunk (Pool queue), critical
        # prefix bytes (wq/wk pair-0 columns, x^T block 0) first.
        mpool = ctx.enter_context(tc.tile_pool(name="mask", bufs=2))
        mk_tiles = {}

        def issue_mask(qc):
            q0 = qc * QCHUNK
            mk = mpool.tile([P, NKT, QCHUNK], BF16, tag="mk", name=f"mk{qc}")
            nc.gpsimd.dma_start(
                out=mk,
                in_=negmt[:, q0:q0 + QCHUNK].rearrange(
                    "(j p) q -> p j q", p=P))
            mk_tiles[qc] = mk

        issue_mask(0)
        issue_mask(1)
        # ACT queue: Q/K weights (pair-0 columns first)
        nc.scalar.dma_start(
            out=wq_all[:, :, 0:P],
            in_=wqt[:, 0:P].rearrange("(ci p) c -> p ci c", p=P))
        nc.scalar.dma_start(
            out=wk_all[:, :, 0:P],
            in_=wkt[:, 0:P].rearrange("(ci p) c -> p ci c", p=P))
        nc.scalar.dma_start(
            out=wv_all, in_=wvt.rearrange("(ci p) c -> p ci c", p=P))
        nc.scalar.dma_start(
            out=wq_all[:, :, P:CH],
            in_=wqt[:, P:CH].rearrange("(ci p) c -> p ci c", p=P))
        nc.scalar.dma_start(
            out=wk_all[:, :, P:CH],
            in_=wkt[:, P:CH].rearrange("(ci p) c -> p ci c", p=P))
        # SP queue: x^T in q-block pieces (consumed in this order)
        nc.sync.dma_start(
            out=xt_all[:, :, 0:QCHUNK],
            in_=xT[:, 0:QCHUNK].rearrange("(ci p) n -> p ci n", p=P))
        nc.sync.dma_start(
            out=xt_all[:, :, QCHUNK:2 * QCHUNK],
            in_=xT[:, QCHUNK:2 * QCHUNK].rearrange("(ci p) n -> p ci n", p=P))
        nc.sync.dma_start(
            out=xt_all[:, :, 2 * QCHUNK:N],
            in_=xT[:, 2 * QCHUNK:N].rearrange("(ci p) n -> p ci n", p=P))
        nc.scalar.dma_start(
            out=wp_all, in_=wpt.rearrange("(ct p) c -> p ct c", p=P))

        # ---- pointwise pools ----
        p_pool = ctx.enter_context(tc.tile_pool(name="pexp", bufs=3))
        pm_pool = ctx.enter_context(tc.tile_pool(name="pmask", bufs=12))
        epi = ctx.enter_context(tc.tile_pool(name="epi", bufs=6))
        otsb_pool = ctx.enter_context(tc.tile_pool(name="otsb", bufs=4))
        outsb_pool = ctx.enter_context(tc.tile_pool(name="outsb", bufs=3))

        # ---- projection units (one psum slot + eviction each) ----
        v_emitted = set()

        def unit_q(m, n, w_all, dst):
            ps = o_pool.tile([P, QCHUNK], F32, tag="ot", name=f"pj{m}_{n}")
            for ci in range(CIN_T):
                nc.tensor.matmul(
                    ps,
                    w_all[:, ci, m * P:(m + 1) * P],
                    xt_all[:, ci, n * QCHUNK:(n + 1) * QCHUNK],
                    start=(ci == 0), stop=(ci == CIN_T - 1))
            dt = dst[m]
            nsl = slice(n * QCHUNK, (n + 1) * QCHUNK)
            # both packed heads in one copy (GPSIMD cannot read PSUM)
            nc.vector.tensor_copy(dt[:, nsl], ps)

        def unit_v(j):
            ps = o_pool.tile([P, CH], F32, tag="ot", name=f"vps{j}")
            for ci in range(CIN_T):
                nc.tensor.matmul(
                    ps,
                    xt_all[:, ci, j * P:(j + 1) * P],
                    wv_all[:, ci, :],
                    start=(ci == 0), stop=(ci == CIN_T - 1))
            nc.gpsimd.memset(vp_sb[j], 1.0)
            nc.vector.tensor_copy(
                vp_sb[j][:, :, 0:D],
                ps.rearrange("p (h d) -> p h d", h=HL))
            v_emitted.add(j)

        # ---- attention iteration pieces ----
        # iterations come in pairs: two QK+exp (2 k-tiles each) fill one
        # [P, 4, QCHUNK] pexp tile; one mask-mul + one 4-k-tile PV group
        # per pair (halves the DVE instruction count).
        pvq = deque()          # deferred PV groups: (h, g, pm, otiles)
        cur_pexp = [None]

        def emit_qk_exp(qc, h, ktp, otiles):
            m, r = divmod(h, 2)
            rows = slice(r * D, (r + 1) * D)
            q0 = qc * QCHUNK
            half = ktp % 2
            ring = ring_pool.tile([P, 2, QCHUNK], F32, tag="ring",
                                  name=f"ring{qc}_{h}_{ktp}")
            for u in range(2):
                kti = 2 * ktp + u
                nc.tensor.matmul(
                    ring[:, u, :],
                    kt_sb[m][rows, kti * P:(kti + 1) * P],
                    qt_sb[m][rows, q0:q0 + QCHUNK],
                    start=True, stop=True)
            if half == 0:
                cur_pexp[0] = p_pool.tile([P, 4, QCHUNK], BF16, tag="pe",
                                          name=f"pe{qc}_{h}_{ktp}")
            pexp = cur_pexp[0]
            nc.scalar.activation(
                pexp[:, 2 * half:2 * half + 2, :], ring,
                mybir.ActivationFunctionType.Exp)
            if half == 1:
                g = ktp // 2           # 4-k-tile group within the head
                pm = pm_pool.tile([P, 4, QCHUNK], BF16, tag="pm",
                                  name=f"pm{qc}_{h}_{g}")
                nc.vector.tensor_mul(
                    pm, pexp, mk_tiles[qc][:, 4 * g:4 * g + 4, :])
                pvq.append((h, g, pm, otiles))

        def front_ready(qc):
            if qc > 0:
                return True
            g = pvq[0][1]
            return all(4 * g + u in v_emitted for u in range(4))

        # ---- softmax divisions, eager per head-range: heads 0..4 divide
        # as soon as their PV chains complete (mid-qchunk); only head 5
        # remains on the post-stream critical path ----
        osb_store = {}

        def emit_divisions(qc, otiles, h0, nh):
            for s in range(QSUBS):
                otv = otiles[s].rearrange("p (h e) -> p h e", h=HL)
                zrec = epi.tile([P, nh], F32, tag=f"zr{nh}",
                                name=f"zr{qc}_{s}_{h0}")
                nc.vector.reciprocal(zrec, otv[:, h0:h0 + nh, D])
                osb = osb_store.get((qc, s))
                if osb is None:
                    osb = epi.tile([P, HL, D], BF16, tag="osb", bufs=8,
                                   name=f"osb{qc}_{s}")
                    osb_store[(qc, s)] = osb
                zb = bass.AP(
                    tensor=zrec.tensor, offset=zrec.offset,
                    ap=[*zrec.ap, [0, D]])
                nc.vector.tensor_mul(
                    osb[:, h0:h0 + nh, :], otv[:, h0:h0 + nh, 0:D], zb)

        def emit_pv_group(qc):
            h, g, pm, otiles = pvq.popleft()
            for u in range(4):
                kti = 4 * g + u
                for s in range(QSUBS):
                    nc.tensor.matmul(
                        otiles[s][:, h * E:(h + 1) * E],
                        pm[:, u, s * P:(s + 1) * P],
                        vp_sb[kti][:, h, :],
                        start=(kti == 0), stop=(kti == NKT - 1))
            if g == NKT // 4 - 1:
                if h == HL - 2:
                    emit_divisions(qc, otiles, 0, HL - 1)
                elif h == HL - 1:
                    emit_divisions(qc, otiles, HL - 1, 1)

        def epi_stile(qc, s, last=False):
            q0 = qc * QCHUNK
            osf = osb_store[(qc, s)].rearrange("p h d -> p (h d)")
            otp = o_pool.tile([P, CH_T, P], BF16, tag="ot",
                              name=f"otp{qc}_{s}")
            for ct in range(CH_T):
                nc.tensor.transpose(
                    otp[:, ct, :], osf[:, ct * P:(ct + 1) * P], idn)
            otsb = otsb_pool.tile([P, CH_T, P], BF16, tag="otsb",
                                  name=f"otsb{qc}_{s}")
            # ACT: lands in the PE-bound boundary window where exp idles
            nc.scalar.copy(otsb, otp)
            ppss = []
            for cf, (c0, c1) in enumerate(((0, CH), (CH, C))):
                pps = o_pool.tile([P, CH], F32, tag="ot",
                                  name=f"pps{qc}_{s}_{cf}")
                for ct in range(CH_T):
                    nc.tensor.matmul(
                        pps,
                        otsb[:, ct, :],
                        wp_all[:, ct, c0:c1],
                        start=(ct == 0), stop=(ct == CH_T - 1))
                ppss.append(pps)
            ob = outsb_pool.tile([P, C], F32, tag="ob", name=f"ob{qc}_{s}")
            if last:
                nc.scalar.copy(ob[:, 0:CH], ppss[0])
                nc.vector.tensor_copy(ob[:, CH:C], ppss[1])
            else:
                nc.vector.tensor_copy(ob[:, 0:CH], ppss[0])
                nc.vector.tensor_copy(ob[:, CH:C], ppss[1])
            nc.sync.dma_start(
                out=out[q0 + s * P:q0 + (s + 1) * P, :], in_=ob)

        # ---- emission ----
        # prefix: Q/K projections for head pair 0, q/k block 0
        unit_q(0, 0, wq_all, qt_sb)
        unit_q(0, 0, wk_all, kt_sb)
        make_identity(nc, idn)

        def run_qchunk(qc, window, window_units, drain_rate):
            otiles = None
            wi = 0
            drain_credit = 0.0
            for it in range(NIT):
                h, ktp = divmod(it, NKT // 2)
                if it == window:
                    otiles = [o_pool.tile([P, HL * E], F32, tag="ot",
                                          name=f"otile{qc}_{s_}")
                              for s_ in range(QSUBS)]
                    old = [(g[0], g[1], g[2]) for g in pvq]
                    pvq.clear()
                    for gh, gg, gpm in old:
                        pvq.append((gh, gg, gpm, otiles))
                emit_qk_exp(qc, h, ktp, otiles)
                if it < window:
                    due = ((it + 1) * len(window_units) + window - 1) // window
                    while wi < min(due, len(window_units)):
                        window_units[wi]()
                        wi += 1
                else:
                    drain_credit = min(drain_credit + drain_rate, 1.5)
                    while (drain_credit >= 1.0 and len(pvq) > 1
                           and front_ready(qc)):
                        emit_pv_group(qc)
                        drain_credit -= 1.0
            while pvq:
                emit_pv_group(qc)
            return otiles

        pending = None
        for qc in range(NQC):
            if qc == 0:
                # 32 units: V(16), K(0,1..3), K(1,*), K(2,*), Q(1,0),
                # Q(2,0), Q(*,1).  Deadlines: K(0,n) before iter 2n;
                # K/Q(1,*) before iter 16; K/Q(2,*) before iter 32;
                # V(j) before the h0 drain; Q(*,1) before qc1.
                window = 24
                window_units = (
                    [lambda: unit_v(0),
                     lambda: unit_q(0, 1, wk_all, kt_sb),
                     lambda: unit_v(1),
                     lambda: unit_q(0, 2, wk_all, kt_sb),
                     lambda: unit_v(2),
                     lambda: unit_q(0, 3, wk_all, kt_sb),
                     lambda: unit_v(3),
                     lambda: unit_v(4),
                     lambda: unit_q(1, 0, wq_all, qt_sb)]
                    + [lambda n=n: unit_q(1, n, wk_all, kt_sb)
                       for n in range(NQC)]
                    + [lambda j=j: unit_v(j) for j in (5, 6, 7, 8)]
                    + [lambda: unit_q(2, 0, wq_all, qt_sb)]
                    + [lambda n=n: unit_q(2, n, wk_all, kt_sb)
                       for n in range(NQC)]
                    + [lambda j=j: unit_v(j) for j in range(9, NKT)]
                    + [lambda m=m: unit_q(m, 1, wq_all, qt_sb)
                       for m in range(MP)])
                drain_rate = 1.0
            else:
                window = 16
                if qc + 1 < NQC:
                    issue_mask(qc + 1)
                pqc = pending
                window_units = []
                for s in range(QSUBS):
                    window_units.append(
                        lambda s=s, p=pqc: epi_stile(p, s))
                    if qc < NQC - 1 and s < MP:
                        window_units.append(
                            lambda m=s, n=qc + 1: unit_q(m, n, wq_all, qt_sb))
                drain_rate = 0.8
            run_qchunk(qc, window, window_units, drain_rate)
            pending = qc

        # ---- final epilogue: stage-parallel across the four s-tiles so
        # the post-exp tail is short; ACT and DVE split the copies ----
        pqc = pending
        otps, otsbs = [], []
        for s in range(QSUBS):
            osf = osb_store[(pqc, s)].rearrange("p h d -> p (h d)")
            otp = o_pool.tile([P, CH_T, P], BF16, tag="ot", name=f"fotp{s}")
            for ct in range(CH_T):
                nc.tensor.transpose(
                    otp[:, ct, :], osf[:, ct * P:(ct + 1) * P], idn)
            otps.append(otp)
            otsb = otsb_pool.tile([P, CH_T, P], BF16, tag="otsb",
                                  name=f"fotsb{s}")
            if s % 2 == 0:
                nc.scalar.copy(otsb, otp)
            else:
                nc.vector.tensor_copy(otsb, otp)
            otsbs.append(otsb)
        for s in range(QSUBS):
            ppss = []
            for cf, (c0, c1) in enumerate(((0, CH), (CH, C))):
                pps = o_pool.tile([P, CH], F32, tag="ot", name=f"fpps{s}_{cf}")
                for ct in range(CH_T):
                    nc.tensor.matmul(
                        pps, otsbs[s][:, ct, :], wp_all[:, ct, c0:c1],
                        start=(ct == 0), stop=(ct == CH_T - 1))
                ppss.append(pps)
            ob = outsb_pool.tile([P, C], F32, tag="ob", name=f"fob{s}")
            nc.scalar.copy(ob[:, 0:CH], ppss[0])
            nc.vector.tensor_copy(ob[:, CH:C], ppss[1])
            nc.scalar.dma_start(
                out=out[pqc * QCHUNK + s * P:pqc * QCHUNK + (s + 1) * P, :],
                in_=ob)

    nc.compile()
    return nc


_CACHE = {}


def _get_nc():
    if "nc" not in _CACHE:
        _CACHE["nc"] = build_kernel()
    return _CACHE["nc"]


def kernel(x, mask, Wq, Wkv, Wp, bp):
    x = np.asarray(x, np.float32)
    mask = np.asarray(mask, np.float32)
    Wq = np.asarray(Wq, np.float32)
    Wkv = np.asarray(Wkv, np.float32)
    Wp = np.asarray(Wp, np.float32)
    bp = np.asarray(bp, np.float32)

    nc = _get_nc()
    in_maps = []
    for core in range(NCORES):
        b, hg = divmod(core, HGROUPS)
        rows = slice(hg * CH, (hg + 1) * CH)
        in_maps.append({
            "xT": np.ascontiguousarray(x[b].T.astype(ml_dtypes.bfloat16)),
            "wqt": np.ascontiguousarray(((Wq[rows, :] * SCALE).T).astype(ml_dtypes.bfloat16)),
            "wkt": np.ascontiguousarray(Wkv[rows, :].T.astype(ml_dtypes.bfloat16)),
            "wvt": np.ascontiguousarray(Wkv.T[:, C + hg * CH:C + (hg + 1) * CH].astype(ml_dtypes.bfloat16)),
            "wpt": np.ascontiguousarray(Wp[:, rows].T.astype(ml_dtypes.bfloat16)),
            "negmt": np.ascontiguousarray(
                (1.0 - mask[b].T).astype(ml_dtypes.bfloat16)),
        })

    trace = os.environ.get("KERNEL_TRACE", "0") == "1"
    if os.environ.get("KERNEL_WARMUP", "1") == "1":
        run_bass_kernel_spmd(nc, in_maps, core_ids=list(range(NCORES)),
                             trace=False)
    res = run_bass_kernel_spmd(nc, in_maps, core_ids=list(range(NCORES)),
                               trace=trace)
    kernel.last_results = res

    outs = [res.results[i]["out"] for i in range(NCORES)]
    full = np.empty((B, N, C), np.float32)
    for b in range(B):
        full[b] = outs[2 * b] + outs[2 * b + 1] + bp[None, :]
    return full


# revision 34
# speedup vs baseline: 1.1449x; 1.0708x over previous
"""Distributed Trainium2 Bass kernel for masked multi-head attention.

BASELINE BACKUP (286,170 ns measured) — restore to kernel.py if newer
variants end up slower.

Problem: out = (softmax(scale * x Wq^T (x Wk^T)^T + mask * -1e5) (x Wv^T)) Wp^T + bp
  x [4, 2048, 768], mask [4, 2048, 2048], H=12 heads, D=64.

Sharding (8 cores): core = (batch b, head-group hg) with b = core//2,
hg = core%2 (6 heads each).  Column-parallel Wq/Wkv, row-parallel Wp;
each core produces a partial [2048, 768] output; the host sums the two
head-group partials per batch and adds the bias (the row-parallel
reduction), then stacks batches.
"""

import os
from contextlib import ExitStack

import ml_dtypes
import numpy as np

import sys
import types

try:  # defensive: concourse's trace path imports this; absent on some images
    import antenv.axon_hooks  # noqa: F401
except ImportError:
    try:
        import antenv
        _m = types.ModuleType('antenv.axon_hooks')
        _m._hook = None
        _m.set_axon_ntff_profile_hook = lambda h: setattr(_m, '_hook', h)
        _m.get_axon_ntff_profile_hook = lambda: _m._hook
        sys.modules['antenv.axon_hooks'] = _m
        antenv.axon_hooks = _m
    except ImportError:
        pass

import concourse.bass as bass
import concourse.tile as tile
from concourse import bacc, mybir
from concourse.bass_utils import run_bass_kernel_spmd
from concourse.masks import make_identity

B, N, C, H, D = 4, 2048, 768, 12, 64
SCALE = D ** -0.5
NCORES = 8
HGROUPS = 2
HL = H // HGROUPS          # 6 heads per group
CH = HL * D                # 384 channels per group
P = 128
NKT = N // P               # 16 k tiles
QCHUNK = 512
NQC = N // QCHUNK          # 4 q chunks
QSUBS = QCHUNK // P        # 4
CIN_T = C // P             # 6 input-channel tiles
CH_T = CH // P             # 3 group-channel tiles
E = D + 1                  # head slot width in O psum (64 V cols + 1 ones col)

F32 = mybir.dt.float32
F32R = mybir.dt.float32r
BF16 = mybir.dt.bfloat16


def build_kernel(debug=False):
    nc = bacc.Bacc("TRN2", target_bir_lowering=False, debug=False,
                   num_devices=NCORES)

    xT = nc.dram_tensor("xT", [C, N], BF16, kind="ExternalInput").ap()
    wqt = nc.dram_tensor("wqt", [C, CH], BF16, kind="ExternalInput").ap()
    wkt = nc.dram_tensor("wkt", [C, CH], BF16, kind="ExternalInput").ap()
    wvt = nc.dram_tensor("wvt", [C, CH], BF16, kind="ExternalInput").ap()
    wpt = nc.dram_tensor("wpt", [CH, C], BF16, kind="ExternalInput").ap()
    negmt = nc.dram_tensor("negmt", [N, N], BF16, kind="ExternalInput").ap()
    out = nc.dram_tensor("out", [N, C], F32, kind="ExternalOutput").ap()

    with tile.TileContext(nc) as tc, ExitStack() as ctx:
        persist = ctx.enter_context(tc.tile_pool(name="persist", bufs=1))
        # PSUM pools shared by both phases: "ring" slots are 2 banks each,
        # "ot" slots 1 bank each -> 2*2 + 4*1 = 8 banks total.
        ring_pool = ctx.enter_context(
            tc.tile_pool(name="ring", bufs=2, space="PSUM"))
        o_pool = ctx.enter_context(
            tc.tile_pool(name="opsum", bufs=4, space="PSUM"))

        qt_sb = [persist.tile([P, N], BF16, tag=f"qt{i}", name=f"qt{i}") for i in range(HL)]
        kt_sb = [persist.tile([P, N], BF16, tag=f"kt{i}", name=f"kt{i}") for i in range(HL)]
        vp_sb = [persist.tile([P, HL, E], BF16, tag=f"vp{j}", name=f"vp{j}")
                 for j in range(NKT)]
        wp_sb = [persist.tile([P, C], BF16, tag=f"wp{t}", name=f"wp{t}") for t in range(CH_T)]
        idn = persist.tile([P, P], BF16, tag="idn")

        # ---- phase 1: projections ----
        # Packed input tensors, one DMA each, spread across queues: SP
        # carries x^T (block 0 first), ACT the weights, Pool the masks.
        # (Each dma_start costs ~1us of issue time on its engine.)
        ph1 = ctx.enter_context(tc.tile_pool(name="ph1", bufs=1))
        xt_all = ph1.tile([P, CIN_T, N], BF16, tag="xt", name="xt")
        wq_all = ph1.tile([P, CIN_T, CH], BF16, tag="wq", name="wq")
        wk_all = ph1.tile([P, CIN_T, CH], BF16, tag="wk", name="wk")
        wv_all = ph1.tile([P, CIN_T, CH], BF16, tag="wv", name="wv")
        xt_sb = [xt_all[:, i, :] for i in range(CIN_T)]
        wq_sb = [wq_all[:, i, :] for i in range(CIN_T)]
        wk_sb = [wk_all[:, i, :] for i in range(CIN_T)]
        wv_sb = [wv_all[:, i, :] for i in range(CIN_T)]
        nc.sync.dma_start(
            out=xt_all[:, :, 0:QCHUNK],
            in_=xT[:, 0:QCHUNK].rearrange("(ci p) n -> p ci n", p=P))
        nc.sync.dma_start(
            out=xt_all[:, :, QCHUNK:N],
            in_=xT[:, QCHUNK:N].rearrange("(ci p) n -> p ci n", p=P))
        nc.scalar.dma_start(
            out=wv_all, in_=wvt.rearrange("(ci p) c -> p ci c", p=P))
        nc.scalar.dma_start(
            out=wq_all, in_=wqt.rearrange("(ci p) c -> p ci c", p=P))
        nc.scalar.dma_start(
            out=wk_all, in_=wkt.rearrange("(ci p) c -> p ci c", p=P))
        for t in range(CH_T):
            nc.scalar.dma_start(out=wp_sb[t], in_=wpt[t * P:(t + 1) * P, :])

        for t in (0, 1):
            nc.gpsimd.memset(qt_sb[t][D:P, :], 0.0)
            nc.gpsimd.memset(kt_sb[t][D:P, :], 0.0)

        for j in range(NKT):
            ps = o_pool.tile([P, CH], F32, tag="ot")
            for ci in range(CIN_T):
                nc.tensor.matmul(
                    ps,
                    xt_sb[ci][:, j * P:(j + 1) * P],
                    wv_sb[ci],
                    start=(ci == 0), stop=(ci == CIN_T - 1))
            nc.gpsimd.memset(vp_sb[j], 1.0)
            nc.vector.tensor_copy(
                vp_sb[j][:, :, 0:D],
                ps.rearrange("p (h d) -> p h d", h=HL))

        def emit_qtkt_chunk(m, which, nck, force_ot=False, evict_dve=False):
            dst, w_sb, evict = (
                (qt_sb, wq_sb, "act") if which == 0 else (kt_sb, wk_sb, "dve"))
            if evict_dve:
                evict = "dve"
            if force_ot:
                pool, tg = o_pool, "ot"
            else:
                pool, tg = ((ring_pool, "ring") if nck % 2 == 0
                            else (o_pool, "ot"))
            ps = pool.tile([P, 512], F32, tag=tg,
                           name=f"p1ps{m}_{which}_{nck}")
            for ci in range(CIN_T):
                nc.tensor.matmul(
                    ps,
                    w_sb[ci][:, m * P:(m + 1) * P],
                    xt_sb[ci][:, nck * 512:(nck + 1) * 512],
                    start=(ci == 0), stop=(ci == CIN_T - 1))
            for sub in range(2):
                dtile = dst[2 * m + sub]
                dslice = dtile[0:D, nck * 512:(nck + 1) * 512]
                pslice = ps[sub * D:(sub + 1) * D, :]
                if evict == "act":
                    nc.scalar.copy(dslice, pslice)
                else:
                    nc.vector.tensor_copy(dslice, pslice)

        # Slim phase 1: K for all q-pair columns, but Q only for q-block
        # 0 — Q blocks 1..3 are projected during qchunks 0..2.
        for t in range(2, HL):
            nc.gpsimd.memset(qt_sb[t][D:P, :], 0.0)
            nc.gpsimd.memset(kt_sb[t][D:P, :], 0.0)
        for m in range(HL // 2):
            emit_qtkt_chunk(m, 0, 0)
            for nck in range(N // 512):
                emit_qtkt_chunk(m, 1, nck)
        make_identity(nc, idn)

        # ---- phase 2: attention ----
        mpool = ctx.enter_context(tc.tile_pool(name="mask", bufs=2))
        p_pool = ctx.enter_context(tc.tile_pool(name="pexp", bufs=6))
        pm_pool = ctx.enter_context(tc.tile_pool(name="pmask", bufs=6))
        epi = ctx.enter_context(tc.tile_pool(name="epi", bufs=8))
        ot_pool = ctx.enter_context(tc.tile_pool(name="otsb", bufs=2))
        outsb_pool = ctx.enter_context(tc.tile_pool(name="outsb", bufs=4))

        def make_epilogue(qc, q0, otiles):
            last = (qc == NQC - 1)

            def copy_out(dst, src):
                if last:
                    nc.scalar.copy(dst, src)
                else:
                    nc.vector.tensor_copy(dst, src)

            def epi_fn():
                otsb = ot_pool.tile([P, CH_T, QCHUNK], BF16, tag="otsb",
                                    name=f"otsb{qc}")
                osbs = []
                for s in range(QSUBS):
                    otv = otiles[s].rearrange("p (h e) -> p h e", h=HL)
                    zrec = epi.tile([P, HL], F32, tag="zr", name=f"zr{qc}_{s}")
                    nc.vector.reciprocal(zrec, otv[:, :, D])
                    osb = epi.tile([P, HL, D], BF16, tag="osb",
                                   name=f"osb{qc}_{s}")
                    zb = bass.AP(
                        tensor=zrec.tensor, offset=zrec.offset,
                        ap=[*zrec.ap, [0, D]])
                    nc.vector.tensor_mul(osb, otv[:, :, 0:D], zb)
                    osbs.append(osb)
                for s in range(QSUBS):
                    osf = osbs[s].rearrange("p h d -> p (h d)")
                    otp = o_pool.tile([P, CH_T, P], BF16, tag="ot",
                                      name=f"otp{qc}_{s}")
                    for ct in range(CH_T):
                        nc.tensor.transpose(
                            otp[:, ct, :], osf[:, ct * P:(ct + 1) * P], idn)
                    copy_out(otsb[:, :, s * P:(s + 1) * P], otp)
                for s in range(QSUBS):
                    ppss = []
                    for cf, (c0, c1) in enumerate(((0, CH), (CH, C))):
                        pps = o_pool.tile([P, CH], F32, tag="ot",
                                          name=f"pps{qc}_{s}_{cf}")
                        for ct in range(CH_T):
                            nc.tensor.matmul(
                                pps,
                                otsb[:, ct, s * P:(s + 1) * P],
                                wp_sb[ct][:, c0:c1],
                                start=(ct == 0), stop=(ct == CH_T - 1))
                        ppss.append(pps)
                    ob = outsb_pool.tile([P, C], F32, tag="ob",
                                         name=f"ob{qc}_{s}")
                    copy_out(ob[:, 0:CH], ppss[0])
                    copy_out(ob[:, CH:C], ppss[1])
                    nc.sync.dma_start(
                        out=out[q0 + s * P:q0 + (s + 1) * P, :], in_=ob)
            return epi_fn

        pending_epi = None
        for qc in range(NQC):
            q0 = qc * QCHUNK
            mk = mpool.tile([P, NKT, QCHUNK], BF16, tag="mk")
            nc.gpsimd.dma_start(
                out=mk,
                in_=negmt[:, q0:q0 + QCHUNK].rearrange(
                    "(j p) q -> p j q", p=P))

            otiles = [o_pool.tile([P, HL * E], F32, tag="ot",
                                  name=f"otile{qc}_{s_}")
                      for s_ in range(QSUBS)]

            for h in range(HL):
                if 1 <= h <= 3 and qc + 1 < NQC:
                    # project Q block qc+1, head pair h-1 (DVE eviction:
                    # ACT is the exp pace-setter here)
                    emit_qtkt_chunk(h - 1, 0, qc + 1,
                                    force_ot=True, evict_dve=True)
                kth = kt_sb[h]
                qth = qt_sb[h]
                for ktp in range(NKT // 2):
                    ring = ring_pool.tile([P, 2, QCHUNK], F32, tag="ring")
                    for u in range(2):
                        kti = 2 * ktp + u
                        nc.tensor.matmul(
                            ring[:, u, :],
                            kth[:, kti * P:(kti + 1) * P],
                            qth[:, q0:q0 + QCHUNK],
                            start=True, stop=True)
                    pexp = p_pool.tile([P, 2, QCHUNK], BF16, tag="pe")
                    nc.scalar.activation(
                        pexp, ring, mybir.ActivationFunctionType.Exp)
                    pm = pm_pool.tile([P, 2, QCHUNK], BF16, tag="pm")
                    nc.vector.tensor_mul(
                        pm, pexp, mk[:, 2 * ktp:2 * ktp + 2, :])
                    for u in range(2):
                        kti = 2 * ktp + u
                        for s in range(QSUBS):
                            nc.tensor.matmul(
                                otiles[s][:, h * E:(h + 1) * E],
                                pm[:, u, s * P:(s + 1) * P],
                                vp_sb[kti][:, h, :],
                                start=(kti == 0), stop=(kti == NKT - 1))
                if h == 0 and pending_epi is not None:
                    pending_epi()
                    pending_epi = None
            pending_epi = make_epilogue(qc, q0, otiles)
        pending_epi()

    nc.compile()
    return nc


_CACHE = {}


def _get_nc():
    if "nc" not in _CACHE:
        _CACHE["nc"] = build_kernel()
    return _CACHE["nc"]


def kernel(x, mask, Wq, Wkv, Wp, bp):
    x = np.asarray(x, np.float32)
    mask = np.asarray(mask, np.float32)
    Wq = np.asarray(Wq, np.float32)
    Wkv = np.asarray(Wkv, np.float32)
    Wp = np.asarray(Wp, np.float32)
    bp = np.asarray(bp, np.float32)

    nc = _get_nc()
    in_maps = []
    for core in range(NCORES):
        b, hg = divmod(core, HGROUPS)
        rows = slice(hg * CH, (hg + 1) * CH)
        in_maps.append({
            "xT": np.ascontiguousarray(x[b].T.astype(ml_dtypes.bfloat16)),
            "wqt": np.ascontiguousarray(((Wq[rows, :] * SCALE).T).astype(ml_dtypes.bfloat16)),
            "wkt": np.ascontiguousarray(Wkv[rows, :].T.astype(ml_dtypes.bfloat16)),
            "wvt": np.ascontiguousarray(Wkv.T[:, C + hg * CH:C + (hg + 1) * CH].astype(ml_dtypes.bfloat16)),
            "wpt": np.ascontiguousarray(Wp[:, rows].T.astype(ml_dtypes.bfloat16)),
            "negmt": np.ascontiguousarray(
                (1.0 - mask[b].T).astype(ml_dtypes.bfloat16)),
        })

    trace = os.environ.get("KERNEL_TRACE", "0") == "1"
    if os.environ.get("KERNEL_WARMUP", "1") == "1":
        run_bass_kernel_spmd(nc, in_maps, core_ids=list(range(NCORES)),
                             trace=False)
    res = run_bass_kernel_spmd(nc, in_maps, core_ids=list(range(NCORES)),
                               trace=trace)
    kernel.last_results = res

    outs = [res.results[i]["out"] for i in range(NCORES)]
    full = np.empty((B, N, C), np.float32)
    for b in range(B):
        full[b] = outs[2 * b] + outs[2 * b + 1] + bp[None, :]
    return full


# revision 38
# speedup vs baseline: 1.2085x; 1.0556x over previous
"""Distributed Trainium2 Bass kernel for masked multi-head attention.

BASELINE BACKUP (286,170 ns measured) — restore to kernel.py if newer
variants end up slower.

Problem: out = (softmax(scale * x Wq^T (x Wk^T)^T + mask * -1e5) (x Wv^T)) Wp^T + bp
  x [4, 2048, 768], mask [4, 2048, 2048], H=12 heads, D=64.

Sharding (8 cores): core = (batch b, head-group hg) with b = core//2,
hg = core%2 (6 heads each).  Column-parallel Wq/Wkv, row-parallel Wp;
each core produces a partial [2048, 768] output; the host sums the two
head-group partials per batch and adds the bias (the row-parallel
reduction), then stacks batches.
"""

import os
from contextlib import ExitStack

import ml_dtypes
import numpy as np

import sys
import types

try:  # defensive: concourse's trace path imports this; absent on some images
    import antenv.axon_hooks  # noqa: F401
except ImportError:
    try:
        import antenv
        _m = types.ModuleType('antenv.axon_hooks')
        _m._hook = None
        _m.set_axon_ntff_profile_hook = lambda h: setattr(_m, '_hook', h)
        _m.get_axon_ntff_profile_hook = lambda: _m._hook
        sys.modules['antenv.axon_hooks'] = _m
        antenv.axon_hooks = _m
    except ImportError:
        pass

import concourse.bass as bass
import concourse.tile as tile
from concourse import bacc, mybir
from concourse.bass_utils import run_bass_kernel_spmd
from concourse.masks import make_identity

B, N, C, H, D = 4, 2048, 768, 12, 64
SCALE = D ** -0.5
NCORES = 8
HGROUPS = 2
HL = H // HGROUPS          # 6 heads per group
CH = HL * D                # 384 channels per group
P = 128
NKT = N // P               # 16 k tiles
QCHUNK = 512
NQC = N // QCHUNK          # 4 q chunks
QSUBS = QCHUNK // P        # 4
CIN_T = C // P             # 6 input-channel tiles
CH_T = CH // P             # 3 group-channel tiles
E = D + 1                  # head slot width in O psum (64 V cols + 1 ones col)

F32 = mybir.dt.float32
F32R = mybir.dt.float32r
BF16 = mybir.dt.bfloat16


def build_kernel(debug=False):
    nc = bacc.Bacc("TRN2", target_bir_lowering=False, debug=False,
                   num_devices=NCORES)

    xT = nc.dram_tensor("xT", [C, N], BF16, kind="ExternalInput").ap()
    wqt = nc.dram_tensor("wqt", [C, CH], BF16, kind="ExternalInput").ap()
    wkt = nc.dram_tensor("wkt", [C, CH], BF16, kind="ExternalInput").ap()
    wvt = nc.dram_tensor("wvt", [C, CH], BF16, kind="ExternalInput").ap()
    wpt = nc.dram_tensor("wpt", [CH, C], BF16, kind="ExternalInput").ap()
    negmt = nc.dram_tensor("negmt", [N, N], BF16, kind="ExternalInput").ap()
    out = nc.dram_tensor("out", [N, C], F32, kind="ExternalOutput").ap()

    with tile.TileContext(nc) as tc, ExitStack() as ctx:
        persist = ctx.enter_context(tc.tile_pool(name="persist", bufs=1))
        # PSUM pools shared by both phases: "ring" slots are 2 banks each,
        # "ot" slots 1 bank each -> 2*2 + 4*1 = 8 banks total.
        ring_pool = ctx.enter_context(
            tc.tile_pool(name="ring", bufs=2, space="PSUM"))
        o_pool = ctx.enter_context(
            tc.tile_pool(name="opsum", bufs=4, space="PSUM"))

        qt_sb = [persist.tile([P, N], BF16, tag=f"qt{i}", name=f"qt{i}") for i in range(HL)]
        kt_sb = [persist.tile([P, N], BF16, tag=f"kt{i}", name=f"kt{i}") for i in range(HL)]
        vp_sb = [persist.tile([P, HL, E], BF16, tag=f"vp{j}", name=f"vp{j}")
                 for j in range(NKT)]
        wp_sb = [persist.tile([P, C], BF16, tag=f"wp{t}", name=f"wp{t}") for t in range(CH_T)]
        idn = persist.tile([P, P], BF16, tag="idn")

        # ---- phase 1: projections ----
        # Packed input tensors, one DMA each, spread across queues: SP
        # carries x^T (block 0 first), ACT the weights, Pool the masks.
        # (Each dma_start costs ~1us of issue time on its engine.)
        ph1 = ctx.enter_context(tc.tile_pool(name="ph1", bufs=1))
        xt_all = ph1.tile([P, CIN_T, N], BF16, tag="xt", name="xt")
        wq_all = ph1.tile([P, CIN_T, CH], BF16, tag="wq", name="wq")
        wk_all = ph1.tile([P, CIN_T, CH], BF16, tag="wk", name="wk")
        wv_all = ph1.tile([P, CIN_T, CH], BF16, tag="wv", name="wv")
        xt_sb = [xt_all[:, i, :] for i in range(CIN_T)]
        wq_sb = [wq_all[:, i, :] for i in range(CIN_T)]
        wk_sb = [wk_all[:, i, :] for i in range(CIN_T)]
        wv_sb = [wv_all[:, i, :] for i in range(CIN_T)]
        nc.sync.dma_start(
            out=xt_all[:, :, 0:QCHUNK],
            in_=xT[:, 0:QCHUNK].rearrange("(ci p) n -> p ci n", p=P))
        nc.sync.dma_start(
            out=xt_all[:, :, QCHUNK:N],
            in_=xT[:, QCHUNK:N].rearrange("(ci p) n -> p ci n", p=P))
        nc.scalar.dma_start(
            out=wv_all, in_=wvt.rearrange("(ci p) c -> p ci c", p=P))
        nc.scalar.dma_start(
            out=wq_all, in_=wqt.rearrange("(ci p) c -> p ci c", p=P))
        nc.scalar.dma_start(
            out=wk_all, in_=wkt.rearrange("(ci p) c -> p ci c", p=P))
        for t in range(CH_T):
            nc.scalar.dma_start(out=wp_sb[t], in_=wpt[t * P:(t + 1) * P, :])

        for t in (0, 1):
            nc.gpsimd.memset(qt_sb[t][D:P, :], 0.0)
            nc.gpsimd.memset(kt_sb[t][D:P, :], 0.0)

        for j in range(NKT):
            ps = o_pool.tile([P, CH], F32, tag="ot")
            for ci in range(CIN_T):
                nc.tensor.matmul(
                    ps,
                    xt_sb[ci][:, j * P:(j + 1) * P],
                    wv_sb[ci],
                    start=(ci == 0), stop=(ci == CIN_T - 1))
            nc.gpsimd.memset(vp_sb[j], 1.0)
            nc.vector.tensor_copy(
                vp_sb[j][:, :, 0:D],
                ps.rearrange("p (h d) -> p h d", h=HL))

        def emit_qtkt_chunk(m, which, nck, force_ot=False, evict_dve=False):
            dst, w_sb, evict = (
                (qt_sb, wq_sb, "act") if which == 0 else (kt_sb, wk_sb, "dve"))
            if evict_dve:
                evict = "dve"
            if force_ot:
                pool, tg = o_pool, "ot"
            else:
                pool, tg = ((ring_pool, "ring") if nck % 2 == 0
                            else (o_pool, "ot"))
            ps = pool.tile([P, 512], F32, tag=tg,
                           name=f"p1ps{m}_{which}_{nck}")
            for ci in range(CIN_T):
                nc.tensor.matmul(
                    ps,
                    w_sb[ci][:, m * P:(m + 1) * P],
                    xt_sb[ci][:, nck * 512:(nck + 1) * 512],
                    start=(ci == 0), stop=(ci == CIN_T - 1))
            for sub in range(2):
                dtile = dst[2 * m + sub]
                dslice = dtile[0:D, nck * 512:(nck + 1) * 512]
                pslice = ps[sub * D:(sub + 1) * D, :]
                if evict == "act":
                    nc.scalar.copy(dslice, pslice)
                else:
                    nc.vector.tensor_copy(dslice, pslice)

        def emit_qtkt(m):
            for which in range(2):
                for nck in range(N // 512):
                    emit_qtkt_chunk(m, which, nck)

        for t in range(2, HL):
            nc.gpsimd.memset(qt_sb[t][D:P, :], 0.0)
            nc.gpsimd.memset(kt_sb[t][D:P, :], 0.0)
        emit_qtkt(0)
        emit_qtkt(1)
        emit_qtkt(2)
        make_identity(nc, idn)

        # ---- phase 2: attention ----
        mpool = ctx.enter_context(tc.tile_pool(name="mask", bufs=2))
        p_pool = ctx.enter_context(tc.tile_pool(name="pexp", bufs=6))
        pm_pool = ctx.enter_context(tc.tile_pool(name="pmask", bufs=6))
        epi = ctx.enter_context(tc.tile_pool(name="epi", bufs=8))
        ot_pool = ctx.enter_context(tc.tile_pool(name="otsb", bufs=2))
        outsb_pool = ctx.enter_context(tc.tile_pool(name="outsb", bufs=4))

        # eager divisions for the last qchunk: heads 0..4 divide right
        # after their PV chains complete so the post-stream tail only
        # waits on head 5's division
        last_osbs = {}

        def eager_div(otiles, h0, nh):
            for s in range(QSUBS):
                otv = otiles[s].rearrange("p (h e) -> p h e", h=HL)
                zrec = epi.tile([P, nh], F32, tag=f"zre{nh}",
                                name=f"zre{s}_{h0}")
                nc.vector.reciprocal(zrec, otv[:, h0:h0 + nh, D])
                osb = last_osbs.get(s)
                if osb is None:
                    osb = epi.tile([P, HL, D], BF16, tag="osbL",
                                   name=f"osbL{s}")
                    last_osbs[s] = osb
                zb = bass.AP(
                    tensor=zrec.tensor, offset=zrec.offset,
                    ap=[*zrec.ap, [0, D]])
                nc.vector.tensor_mul(
                    osb[:, h0:h0 + nh, :], otv[:, h0:h0 + nh, 0:D], zb)

        def make_epilogue(qc, q0, otiles):
            last = (qc == NQC - 1)

            def copy_out(dst, src):
                if last:
                    nc.scalar.copy(dst, src)
                else:
                    nc.vector.tensor_copy(dst, src)

            def epi_fn():
                otsb = ot_pool.tile([P, CH_T, QCHUNK], BF16, tag="otsb",
                                    name=f"otsb{qc}")
                if last:
                    osbs = [last_osbs[s] for s in range(QSUBS)]
                else:
                    osbs = []
                for s in range(QSUBS if not last else 0):
                    otv = otiles[s].rearrange("p (h e) -> p h e", h=HL)
                    zrec = epi.tile([P, HL], F32, tag="zr", name=f"zr{qc}_{s}")
                    nc.vector.reciprocal(zrec, otv[:, :, D])
                    osb = epi.tile([P, HL, D], BF16, tag="osb",
                                   name=f"osb{qc}_{s}")
                    zb = bass.AP(
                        tensor=zrec.tensor, offset=zrec.offset,
                        ap=[*zrec.ap, [0, D]])
                    nc.vector.tensor_mul(osb, otv[:, :, 0:D], zb)
                    osbs.append(osb)
                for s in range(QSUBS):
                    osf = osbs[s].rearrange("p h d -> p (h d)")
                    otp = o_pool.tile([P, CH_T, P], BF16, tag="ot",
                                      name=f"otp{qc}_{s}")
                    for ct in range(CH_T):
                        nc.tensor.transpose(
                            otp[:, ct, :], osf[:, ct * P:(ct + 1) * P], idn)
                    copy_out(otsb[:, :, s * P:(s + 1) * P], otp)
                for s in range(QSUBS):
                    ppss = []
                    for cf, (c0, c1) in enumerate(((0, CH), (CH, C))):
                        pps = o_pool.tile([P, CH], F32, tag="ot",
                                          name=f"pps{qc}_{s}_{cf}")
                        for ct in range(CH_T):
                            nc.tensor.matmul(
                                pps,
                                otsb[:, ct, s * P:(s + 1) * P],
                                wp_sb[ct][:, c0:c1],
                                start=(ct == 0), stop=(ct == CH_T - 1))
                        ppss.append(pps)
                    ob = outsb_pool.tile([P, C], F32, tag="ob",
                                         name=f"ob{qc}_{s}")
                    copy_out(ob[:, 0:CH], ppss[0])
                    copy_out(ob[:, CH:C], ppss[1])
                    nc.sync.dma_start(
                        out=out[q0 + s * P:q0 + (s + 1) * P, :], in_=ob)
            return epi_fn

        pending_epi = None
        for qc in range(NQC):
            q0 = qc * QCHUNK
            mk = mpool.tile([P, NKT, QCHUNK], BF16, tag="mk")
            nc.gpsimd.dma_start(
                out=mk,
                in_=negmt[:, q0:q0 + QCHUNK].rearrange(
                    "(j p) q -> p j q", p=P))

            otiles = [o_pool.tile([P, HL * E], F32, tag="ot",
                                  name=f"otile{qc}_{s_}")
                      for s_ in range(QSUBS)]

            for h in range(HL):
                kth = kt_sb[h]
                qth = qt_sb[h]
                for ktp in range(NKT // 2):
                    ring = ring_pool.tile([P, 2, QCHUNK], F32, tag="ring")
                    for u in range(2):
                        kti = 2 * ktp + u
                        nc.tensor.matmul(
                            ring[:, u, :],
                            kth[:, kti * P:(kti + 1) * P],
                            qth[:, q0:q0 + QCHUNK],
                            start=True, stop=True)
                    pexp = p_pool.tile([P, 2, QCHUNK], BF16, tag="pe")
                    nc.scalar.activation(
                        pexp, ring, mybir.ActivationFunctionType.Exp)
                    pm = pm_pool.tile([P, 2, QCHUNK], BF16, tag="pm")
                    nc.vector.tensor_mul(
                        pm, pexp, mk[:, 2 * ktp:2 * ktp + 2, :])
                    for u in range(2):
                        kti = 2 * ktp + u
                        for s in range(QSUBS):
                            nc.tensor.matmul(
                                otiles[s][:, h * E:(h + 1) * E],
                                pm[:, u, s * P:(s + 1) * P],
                                vp_sb[kti][:, h, :],
                                start=(kti == 0), stop=(kti == NKT - 1))
                if qc == NQC - 1:
                    if h == HL - 2:
                        eager_div(otiles, 0, HL - 1)
                    elif h == HL - 1:
                        eager_div(otiles, HL - 1, 1)
                if h == 0 and pending_epi is not None:
                    pending_epi()
                    pending_epi = None
            pending_epi = make_epilogue(qc, q0, otiles)
        pending_epi()

    nc.compile()
    return nc


_CACHE = {}


def _get_nc():
    if "nc" not in _CACHE:
        _CACHE["nc"] = build_kernel()
    return _CACHE["nc"]


def kernel(x, mask, Wq, Wkv, Wp, bp):
    x = np.asarray(x, np.float32)
    mask = np.asarray(mask, np.float32)
    Wq = np.asarray(Wq, np.float32)
    Wkv = np.asarray(Wkv, np.float32)
    Wp = np.asarray(Wp, np.float32)
    bp = np.asarray(bp, np.float32)

    nc = _get_nc()
    in_maps = []
    for core in range(NCORES):
        b, hg = divmod(core, HGROUPS)
        rows = slice(hg * CH, (hg + 1) * CH)
        in_maps.append({
            "xT": np.ascontiguousarray(x[b].T.astype(ml_dtypes.bfloat16)),
            "wqt": np.ascontiguousarray(((Wq[rows, :] * SCALE).T).astype(ml_dtypes.bfloat16)),
            "wkt": np.ascontiguousarray(Wkv[rows, :].T.astype(ml_dtypes.bfloat16)),
            "wvt": np.ascontiguousarray(Wkv.T[:, C + hg * CH:C + (hg + 1) * CH].astype(ml_dtypes.bfloat16)),
            "wpt": np.ascontiguousarray(Wp[:, rows].T.astype(ml_dtypes.bfloat16)),
            "negmt": np.ascontiguousarray(
                (1.0 - mask[b].T).astype(ml_dtypes.bfloat16)),
        })

    trace = os.environ.get("KERNEL_TRACE", "0") == "1"
    if os.environ.get("KERNEL_WARMUP", "1") == "1":
        run_bass_kernel_spmd(nc, in_maps, core_ids=list(range(NCORES)),
                             trace=False)
    res = run_bass_kernel_spmd(nc, in_maps, core_ids=list(range(NCORES)),
                               trace=trace)
    kernel.last_results = res

    outs = [res.results[i]["out"] for i in range(NCORES)]
    full = np.empty((B, N, C), np.float32)
    for b in range(B):
        full[b] = outs[2 * b] + outs[2 * b + 1] + bp[None, :]
    return full
